# revision 1
# baseline (speedup 1.0000x reference)
"""EngramV2 Trainium2 Bass kernel (8-core SPMD, data-parallel over B/T).

Sharding: 8 token shards of 1024 tokens = (batch b, half h) for b in 0..3,
h in 0..1.  Tables + weights replicated per core.  The depthwise conv3 halo
across the two T-halves of a batch is resolved on the host: each core also
outputs its pre-conv boundary y columns, and the host adds the two
cross-shard contribution terms during unsharding.

Per-core pipeline (N=1024 tokens, 8 tiles of 128):
  h_norm = LN(clip(hidden))                     [t,o] layout
  per n-gram n: gather emb rows, LN -> e_norm, PE-transpose -> e^T (fp32r)
    per branch m: k = e^T.T @ W_K[m,n].T via fp32r matmuls (PSUM [t,o])
      s1 = sum_o h_norm*k   (DVE scalar_tensor_tensor accum)
      s2 = sum_o k^2        (ACT Square accum_out)
    s3 = sum_o k = e_norm . colsum(W)  via tiny PE matvec
    score = s1 * rsqrt(s2/D - (s3/D)^2 + eps) / sqrt(D)  (mean-term of
      LN(k) cancels because sum(h_norm) == 0)
    gate = mean_m sigmoid(clip(score)); fused += gate*w_n*emb (re-gathered)
  fused /= gate_sum; transpose -> fused^T; y^T = W_V fp32r matmul [d,t]
  conv3 along free dim t (+bias); edge columns exported for host halo fix.
"""

import os
import sys

for _p in ("/opt/trn_rl_repo",):
    if os.path.isdir(_p) and _p not in sys.path:
        sys.path.insert(0, _p)

import numpy as np

import concourse.bass as bass
import concourse.bacc as bacc
import concourse.mybir as mybir
import concourse.tile as tile
from concourse.bass_utils import run_bass_kernel_spmd
from concourse.masks import make_identity

B, T, D = 4, 2048, 1024
NGRAM, BRANCHES, BUCKET = 3, 4, 100000
N_CORES = 8
NTOK = (B * T) // N_CORES          # 1024 tokens per core
NT = NTOK // 128                   # 8 token tiles
KC = D // 128                      # 8 k-chunks
LN_EPS = 1e-5
INV_SQRT_D = 1.0 / 32.0

F32 = mybir.dt.float32
F32R = mybir.dt.float32r
I32 = mybir.dt.int32

AL = mybir.AluOpType
AF = mybir.ActivationFunctionType


def build_program(reps: int = 1):
    """Build the SPMD Bass program.  reps>1 wraps the body in a static
    For_i loop for differential wall-clock timing."""
    nc = bacc.Bacc("TRN2", target_bir_lowering=False, debug=False,
                   num_devices=N_CORES)

    hid = nc.dram_tensor("hid", [NTOK, D], F32, kind="ExternalInput")
    bks = nc.dram_tensor("bks", [NGRAM, NTOK], I32, kind="ExternalInput")
    tabs = [nc.dram_tensor(f"tab{n}", [BUCKET, D], F32, kind="ExternalInput")
            for n in range(NGRAM)]
    wkt = nc.dram_tensor("wkt", [NGRAM, BRANCHES, D, D], F32R, kind="ExternalInput")
    ws = nc.dram_tensor("ws", [NGRAM, D, BRANCHES], F32R, kind="ExternalInput")
    wvt = nc.dram_tensor("wvt", [D, D], F32R, kind="ExternalInput")
    cp = nc.dram_tensor("cp", [D, 4], F32, kind="ExternalInput")
    outT = nc.dram_tensor("outT", [D, NTOK], F32, kind="ExternalOutput")
    edges = nc.dram_tensor("edges", [D, 2], F32, kind="ExternalOutput")

    with tile.TileContext(nc) as tc:
        from contextlib import ExitStack
        with ExitStack() as ctx:
            big = ctx.enter_context(tc.tile_pool(name="big", bufs=1))
            tptp = ctx.enter_context(tc.tile_pool(name="tptp", bufs=2))
            wkp = ctx.enter_context(tc.tile_pool(name="wkp", bufs=2))
            work = ctx.enter_context(tc.tile_pool(name="work", bufs=2))
            small = ctx.enter_context(tc.tile_pool(name="small", bufs=1))
            sm2 = ctx.enter_context(tc.tile_pool(name="sm2", bufs=2))
            mmp = ctx.enter_context(tc.tile_pool(name="mmp", bufs=4, space="PSUM"))
            tpp = ctx.enter_context(tc.tile_pool(name="tpp", bufs=2, space="PSUM"))
            s3p = ctx.enter_context(tc.tile_pool(name="s3p", bufs=2, space="PSUM"))

            def body():
                # ---- persistent big tiles (32KB/partition each) ----
                hnorm = big.tile([128, NT * D], F32)     # h_norm, [t,o] per tile
                fused = big.tile([128, NT * D], F32)     # gate-weighted emb sum

                # ---- constants ----
                ident = small.tile([128, 128], F32)
                make_identity(nc, ident[:])
                eps_ln = small.tile([128, 1], F32)
                nc.any.memset(eps_ln[:], LN_EPS)
                cp_sb = small.tile([128, 8 * 4], F32)    # conv w0,w1,w2,b per d-tile
                nc.sync.dma_start(
                    out=cp_sb[:].rearrange("p (a c) -> p a c", c=4),
                    in_=cp[:, :].rearrange("(a p) c -> p a c", p=128))
                idx_all = small.tile([128, NGRAM * NT], I32)
                nc.sync.dma_start(
                    out=idx_all[:].rearrange("p (n a) -> p n a", a=NT),
                    in_=bks[:, :].rearrange("n (a p) -> p n a", p=128))
                ws_sb = small.tile([128, NGRAM * KC * BRANCHES], F32R)
                nc.sync.dma_start(
                    out=ws_sb[:].rearrange("p (n k m) -> p n k m", k=KC, m=BRANCHES),
                    in_=ws[:, :, :].rearrange("n (k p) m -> p n k m", p=128))

                # ---- stat buffers ----
                s1h = small.tile([128, NT * 8], F32)   # col (t*2+oh)*4+m
                s2h = small.tile([128, NT * 8], F32)
                gs = small.tile([128, NT], F32)        # gate_sum per tile col
                # batched LN stat columns (per 8-tile group)
                lsum = small.tile([128, NT], F32)
                lq = small.tile([128, 2 * NT], F32)    # (t, half)
                lrstd = small.tile([128, NT], F32)
                lnegmr = small.tile([128, NT], F32)

                def ln_stats_tile(src_ap, t):
                    """Big reduction passes for tile t -> column buffers."""
                    nc.vector.tensor_reduce(out=lsum[:, t:t + 1], in_=src_ap,
                                            op=AL.add, axis=mybir.AxisListType.X)
                    for half in range(2):
                        asc = sm2.tile([128, 512], F32, tag="asc")
                        nc.scalar.activation(
                            out=asc[:], in_=src_ap[:, half * 512:(half + 1) * 512],
                            func=AF.Square,
                            accum_out=lq[:, t * 2 + half:t * 2 + half + 1])

                def ln_finalize():
                    """Batched [128,NT] post-processing -> lrstd, lnegmr."""
                    qsum = sm2.tile([128, NT], F32, tag="qsum")
                    nc.vector.tensor_add(
                        qsum[:],
                        lq[:].rearrange("p (t h) -> p t h", h=2)[:, :, 0],
                        lq[:].rearrange("p (t h) -> p t h", h=2)[:, :, 1])
                    mean = sm2.tile([128, NT], F32, tag="mean")
                    nc.vector.tensor_scalar_mul(mean[:], lsum[:], 1.0 / D)
                    m2 = sm2.tile([128, NT], F32, tag="m2")
                    nc.vector.tensor_mul(m2[:], mean[:], mean[:])
                    var = sm2.tile([128, NT], F32, tag="var")
                    nc.vector.scalar_tensor_tensor(
                        out=var[:], in0=qsum[:], scalar=1.0 / D, in1=m2[:],
                        op0=AL.mult, op1=AL.subtract)
                    std = sm2.tile([128, NT], F32, tag="std")
                    nc.scalar.activation(out=std[:], in_=var[:], func=AF.Sqrt,
                                         bias=eps_ln[:, :1], scale=1.0)
                    nc.vector.reciprocal(lrstd[:], std[:])
                    nc.vector.scalar_tensor_tensor(
                        out=lnegmr[:], in0=mean[:], scalar=-1.0, in1=lrstd[:],
                        op0=AL.mult, op1=AL.mult)

                def transpose_tile(src_ap, t, tpT):
                    """PE-transpose [128t,1024d] -> tpT slabs, batched copies."""
                    for g in range(2):           # groups of 4 k-chunks
                        tps4 = tpp.tile([128, 512], F32, space="PSUM", tag="tps4")
                        for j in range(4):
                            k = g * 4 + j
                            nc.tensor.transpose(
                                out=tps4[:, j * 128:(j + 1) * 128],
                                in_=src_ap[:, k * 128:(k + 1) * 128],
                                identity=ident[:])
                        dst = tpT[:].rearrange("p (k t) -> p k t", t=NTOK)[
                            :, g * 4:(g + 1) * 4, t * 128:(t + 1) * 128]
                        nc.vector.tensor_copy(out=dst, in_=tps4[:])

                def gather_tile(n, t, dst_ap):
                    nc.gpsimd.indirect_dma_start(
                        out=dst_ap, out_offset=None, in_=tabs[n][:],
                        in_offset=bass.IndirectOffsetOnAxis(
                            ap=idx_all[:, n * NT + t:n * NT + t + 1], axis=0))

                def ln_finalize_tile(t):
                    """Per-tile [128,1] finalize -> (rstd, negmr) tiles."""
                    qsum = sm2.tile([128, 1], F32, tag="qsum1")
                    nc.vector.tensor_add(qsum[:], lq[:, t * 2:t * 2 + 1],
                                         lq[:, t * 2 + 1:t * 2 + 2])
                    mean = sm2.tile([128, 1], F32, tag="mean1")
                    nc.vector.tensor_scalar_mul(mean[:], lsum[:, t:t + 1], 1.0 / D)
                    m2 = sm2.tile([128, 1], F32, tag="m21")
                    nc.vector.tensor_mul(m2[:], mean[:], mean[:])
                    var = sm2.tile([128, 1], F32, tag="var1")
                    nc.vector.scalar_tensor_tensor(
                        out=var[:], in0=qsum[:], scalar=1.0 / D, in1=m2[:],
                        op0=AL.mult, op1=AL.subtract)
                    std = sm2.tile([128, 1], F32, tag="std1")
                    nc.scalar.activation(out=std[:], in_=var[:], func=AF.Sqrt,
                                         bias=eps_ln[:, :1], scale=1.0)
                    rstd = sm2.tile([128, 1], F32, tag="rstd1")
                    nc.vector.reciprocal(rstd[:], std[:])
                    negmr = sm2.tile([128, 1], F32, tag="negmr1")
                    nc.vector.scalar_tensor_tensor(
                        out=negmr[:], in0=mean[:], scalar=-1.0, in1=rstd[:],
                        op0=AL.mult, op1=AL.mult)
                    return rstd, negmr

                def prep_s3(n, t, tpT, s3bn):
                    ps3 = s3p.tile([128, BRANCHES], F32, space="PSUM")
                    for k in range(KC):
                        nc.tensor.matmul(
                            out=ps3[:],
                            lhsT=tpT[:, k * NTOK + t * 128:k * NTOK + (t + 1) * 128],
                            rhs=ws_sb[:, (n * KC + k) * BRANCHES:(n * KC + k + 1) * BRANCHES],
                            start=(k == 0), stop=(k == KC - 1))
                    nc.scalar.copy(out=s3bn[:, t * 4:(t + 1) * 4], in_=ps3[:])

                def ngram_prep(n, tpT, pipelined):
                    """Gather + LN + transpose + s3 matvec for all tiles of n.

                    pipelined=True: per-tile finalize, each tile flows gather
                    -> stats -> apply -> transpose with no cross-tile barrier
                    (keeps PE fed at kernel start).  pipelined=False: batched
                    [128,NT] finalize (fewer small ops; embeddings gathered
                    twice since 8 tiles can't be held across the barrier)."""
                    s3bn = tptp.tile([128, NT * 4], F32, tag="s3b")
                    if pipelined:
                        for t in range(NT):
                            em = work.tile([128, D], F32, tag="em")
                            gather_tile(n, t, em[:])
                            ln_stats_tile(em[:], t)
                            rstd, negmr = ln_finalize_tile(t)
                            en = work.tile([128, D], F32, tag="en")
                            nc.scalar.activation(out=en[:], in_=em[:],
                                                 func=AF.Identity,
                                                 bias=negmr[:, :1],
                                                 scale=rstd[:, :1])
                            transpose_tile(en[:], t, tpT)
                            prep_s3(n, t, tpT, s3bn)
                    else:
                        for t in range(NT):
                            em = work.tile([128, D], F32, tag="em")
                            gather_tile(n, t, em[:])
                            ln_stats_tile(em[:], t)
                        ln_finalize()
                        for t in range(NT):
                            em = work.tile([128, D], F32, tag="em")
                            gather_tile(n, t, em[:])
                            en = work.tile([128, D], F32, tag="en")
                            nc.scalar.activation(out=en[:], in_=em[:],
                                                 func=AF.Identity,
                                                 bias=lnegmr[:, t:t + 1],
                                                 scale=lrstd[:, t:t + 1])
                            transpose_tile(en[:], t, tpT)
                            prep_s3(n, t, tpT, s3bn)
                    return s3bn

                # ---- phase 0: first gather/prep interleaved with h_norm ----
                # n=0 prep first so PE gets work early; h_norm tiles follow
                # (only needed once the first branch psums land).
                tpT_cur = tptp.tile([128, KC * NTOK], F32R, tag="tpT")
                s3b_cur = ngram_prep(0, tpT_cur, pipelined=True)
                for t in range(NT):
                    ht = work.tile([128, D], F32, tag="em")
                    nc.sync.dma_start(out=ht[:], in_=hid[t * 128:(t + 1) * 128, :])
                    hslice = hnorm[:, t * D:(t + 1) * D]
                    nc.vector.tensor_scalar(out=hslice, in0=ht[:], scalar1=10.0,
                                            scalar2=-10.0, op0=AL.min, op1=AL.max)
                    ln_stats_tile(hslice, t)
                ln_finalize()
                for t in range(NT):
                    hslice = hnorm[:, t * D:(t + 1) * D]
                    nc.scalar.activation(out=hslice, in_=hslice,
                                         func=AF.Identity,
                                         bias=lnegmr[:, t:t + 1],
                                         scale=lrstd[:, t:t + 1])

                # ---- n-gram loop ----
                for n in range(NGRAM):
                    wn4 = (1.0 + 0.3 * n) / BRANCHES
                    tpT = tpT_cur
                    s3b_n = s3b_cur

                    # branch matmuls + fused reductions; the NEXT n's prep is
                    # emitted mid-loop so its DMA/DVE/ACT work overlaps this
                    # n's matmul tail (separate tpT buffer breaks the WAR).
                    for m in range(BRANCHES):
                        if m == 2 and n + 1 < NGRAM:
                            tpT_cur = tptp.tile([128, KC * NTOK], F32R, tag="tpT")
                            s3b_cur = ngram_prep(n + 1, tpT_cur, pipelined=False)
                        for oh in range(2):
                            wk = wkp.tile([128, KC * 512], F32R, tag="wk")
                            nc.sync.dma_start(
                                out=wk[:].rearrange("p (k o) -> p k o", o=512),
                                in_=wkt[n, m, :, oh * 512:(oh + 1) * 512]
                                .rearrange("(k p) o -> p k o", p=128))
                            for t in range(NT):
                                pk = mmp.tile([128, 512], F32, space="PSUM")
                                for k in range(KC):
                                    nc.tensor.matmul(
                                        out=pk[:],
                                        lhsT=tpT[:, k * NTOK + t * 128:k * NTOK + (t + 1) * 128],
                                        rhs=wk[:, k * 512:(k + 1) * 512],
                                        start=(k == 0), stop=(k == KC - 1))
                                col = (t * 2 + oh) * 4 + m
                                dsc = sm2.tile([128, 512], F32, tag="dsc")
                                nc.vector.scalar_tensor_tensor(
                                    out=dsc[:], in0=pk[:], scalar=1.0,
                                    in1=hnorm[:, t * D + oh * 512:t * D + oh * 512 + 512],
                                    op0=AL.mult, op1=AL.mult,
                                    accum_out=s1h[:, col:col + 1])
                                asc = sm2.tile([128, 512], F32, tag="asc")
                                nc.scalar.activation(
                                    out=asc[:], in_=pk[:], func=AF.Square,
                                    accum_out=s2h[:, col:col + 1])

                    # scores + gates, batched over all tiles: cols are (t,m)
                    s1v = s1h[:].rearrange("p (t h m) -> p t h m", h=2, m=4)
                    s2v = s2h[:].rearrange("p (t h m) -> p t h m", h=2, m=4)
                    s1 = sm2.tile([128, NT * 4], F32, tag="s1")
                    nc.vector.tensor_add(
                        s1[:].rearrange("p (t m) -> p t m", m=4),
                        s1v[:, :, 0, :], s1v[:, :, 1, :])
                    s2 = sm2.tile([128, NT * 4], F32, tag="s2")
                    nc.vector.tensor_add(
                        s2[:].rearrange("p (t m) -> p t m", m=4),
                        s2v[:, :, 0, :], s2v[:, :, 1, :])
                    mk = sm2.tile([128, NT * 4], F32, tag="mk")
                    nc.vector.tensor_scalar_mul(mk[:], s3b_n[:], 1.0 / D)
                    mk2 = sm2.tile([128, NT * 4], F32, tag="mk2")
                    nc.vector.tensor_mul(mk2[:], mk[:], mk[:])
                    var = sm2.tile([128, NT * 4], F32, tag="kvar")
                    nc.vector.scalar_tensor_tensor(
                        out=var[:], in0=s2[:], scalar=1.0 / D, in1=mk2[:],
                        op0=AL.mult, op1=AL.subtract)
                    std = sm2.tile([128, NT * 4], F32, tag="kstd")
                    nc.scalar.activation(out=std[:], in_=var[:], func=AF.Sqrt,
                                         bias=eps_ln[:, :1], scale=1.0)
                    rk = sm2.tile([128, NT * 4], F32, tag="rk")
                    nc.vector.reciprocal(rk[:], std[:])
                    sc = sm2.tile([128, NT * 4], F32, tag="sc")
                    nc.vector.tensor_mul(sc[:], s1[:], rk[:])
                    scc = sm2.tile([128, NT * 4], F32, tag="scc")
                    nc.vector.tensor_scalar(out=scc[:], in0=sc[:],
                                            scalar1=INV_SQRT_D, scalar2=10.0,
                                            op0=AL.mult, op1=AL.min)
                    scc2 = sm2.tile([128, NT * 4], F32, tag="scc2")
                    nc.vector.tensor_scalar_max(scc2[:], scc[:], -10.0)
                    sg = sm2.tile([128, NT * 4], F32, tag="sg")
                    nc.scalar.activation(out=sg[:], in_=scc2[:], func=AF.Sigmoid)
                    gwv = sm2.tile([128, NT], F32, tag="gwv")
                    nc.vector.tensor_reduce(
                        out=gwv[:], in_=sg[:].rearrange("p (t m) -> p t m", m=4),
                        op=AL.add, axis=mybir.AxisListType.X)
                    gwv2 = sm2.tile([128, NT], F32, tag="gwv2")
                    nc.vector.tensor_scalar_mul(gwv2[:], gwv[:], wn4)
                    if n == 0:
                        nc.vector.tensor_copy(out=gs[:], in_=gwv2[:])
                    else:
                        nc.vector.tensor_add(gs[:], gs[:], gwv2[:])

                    # fused accumulation (emb re-gathered per tile)
                    for t in range(NT):
                        em2 = work.tile([128, D], F32, tag="em2")
                        gather_tile(n, t, em2[:])
                        fslice = fused[:, t * D:(t + 1) * D]
                        if n == 0:
                            nc.scalar.mul(out=fslice, in_=em2[:],
                                          mul=gwv2[:, t:t + 1])
                        else:
                            nc.vector.scalar_tensor_tensor(
                                out=fslice, in0=em2[:], scalar=gwv2[:, t:t + 1],
                                in1=fslice, op0=AL.mult, op1=AL.add)

                # ---- fused /= gate_sum; transpose fused ----
                gsp = sm2.tile([128, NT], F32, tag="gsp")
                nc.vector.tensor_scalar_add(gsp[:], gs[:], 1e-8)
                rgs = sm2.tile([128, NT], F32, tag="rgs")
                nc.vector.reciprocal(rgs[:], gsp[:])
                ftT = tptp.tile([128, KC * NTOK], F32R, tag="tpT")
                for t in range(NT):
                    fslice = fused[:, t * D:(t + 1) * D]
                    nc.scalar.mul(out=fslice, in_=fslice, mul=rgs[:, t:t + 1])
                    transpose_tile(fslice, t, ftT)

                # ---- W_V matmul + conv3 ----
                for dp in range(KC):
                    wv = wkp.tile([128, KC * 128], F32R, tag="wv")
                    nc.sync.dma_start(
                        out=wv[:].rearrange("p (k q) -> p k q", q=128),
                        in_=wvt[:, dp * 128:(dp + 1) * 128]
                        .rearrange("(k p) q -> p k q", p=128))
                    yT = work.tile([128, NTOK], F32, tag="en")
                    for tc2 in range(2):
                        py = mmp.tile([128, 512], F32, space="PSUM", tag="pk")
                        for k in range(KC):
                            nc.tensor.matmul(
                                out=py[:],
                                lhsT=wv[:, k * 128:(k + 1) * 128],
                                rhs=ftT[:, k * NTOK + tc2 * 512:k * NTOK + (tc2 + 1) * 512],
                                start=(k == 0), stop=(k == KC - 1))
                        nc.scalar.copy(out=yT[:, tc2 * 512:(tc2 + 1) * 512], in_=py[:])
                    # conv3 along t (free dim); per-channel weights per-partition
                    w0 = cp_sb[:, dp * 4 + 0:dp * 4 + 1]
                    w1 = cp_sb[:, dp * 4 + 1:dp * 4 + 2]
                    w2 = cp_sb[:, dp * 4 + 2:dp * 4 + 3]
                    bb = cp_sb[:, dp * 4 + 3:dp * 4 + 4]
                    co = work.tile([128, NTOK], F32, tag="em2")
                    nc.scalar.activation(out=co[:], in_=yT[:], func=AF.Identity,
                                         bias=bb, scale=w1)
                    nc.vector.scalar_tensor_tensor(
                        out=co[:, 1:NTOK], in0=yT[:, 0:NTOK - 1], scalar=w0,
                        in1=co[:, 1:NTOK], op0=AL.mult, op1=AL.add)
                    nc.vector.scalar_tensor_tensor(
                        out=co[:, 0:NTOK - 1], in0=yT[:, 1:NTOK], scalar=w2,
                        in1=co[:, 0:NTOK - 1], op0=AL.mult, op1=AL.add)
                    nc.sync.dma_start(out=outT[dp * 128:(dp + 1) * 128, :], in_=co[:])
                    nc.sync.dma_start(out=edges[dp * 128:(dp + 1) * 128, 0:1],
                                      in_=yT[:, 0:1])
                    nc.sync.dma_start(out=edges[dp * 128:(dp + 1) * 128, 1:2],
                                      in_=yT[:, NTOK - 1:NTOK])

            if reps == 1:
                body()
            else:
                with tc.For_i(0, reps, 1):
                    body()

    nc.compile()
    return nc


def prep_in_maps(token_ids, hidden, buckets, tables, W_K, W_V, conv_w, conv_b):
    """Host-side shard + weight-layout prep.  Returns per-core input maps."""
    hidden = np.asarray(hidden, dtype=np.float32)
    buckets = np.ascontiguousarray(np.asarray(buckets).astype(np.int32))
    tables = np.ascontiguousarray(np.asarray(tables, dtype=np.float32))
    W_K = np.asarray(W_K, dtype=np.float32)
    W_V = np.asarray(W_V, dtype=np.float32)
    conv_w = np.asarray(conv_w, dtype=np.float32)
    conv_b = np.asarray(conv_b, dtype=np.float32)

    # weight layouts
    wkt = np.ascontiguousarray(
        np.transpose(W_K, (1, 0, 3, 2)))            # [n, m, d, o] = W_K[m,n].T
    ws = np.ascontiguousarray(
        np.transpose(W_K.sum(axis=2), (1, 2, 0)))   # [n, d, m]
    wvt = np.ascontiguousarray(W_V.T)               # [d, d']
    cp = np.ascontiguousarray(
        np.stack([conv_w[:, 0, 0], conv_w[:, 0, 1], conv_w[:, 0, 2], conv_b],
                 axis=1))                           # [D, 4]

    hs = hidden.reshape(B, 2, NTOK, D)              # (b, half, t, d)
    bs = buckets.reshape(NGRAM, B, 2, NTOK)

    in_maps = []
    for c in range(N_CORES):
        b, h = divmod(c, 2)
        m = {
            "hid": np.ascontiguousarray(hs[b, h]),
            "bks": np.ascontiguousarray(bs[:, b, h]),
            "wkt": wkt, "ws": ws, "wvt": wvt, "cp": cp,
        }
        for n in range(NGRAM):
            m[f"tab{n}"] = tables[n]
        in_maps.append(m)
    return in_maps


def assemble_output(results, conv_w):
    """Gather per-core outputs -> (B,T,D), applying the conv halo fixup."""
    conv_w = np.asarray(conv_w, dtype=np.float32)
    w0 = conv_w[:, 0, 0]
    w2 = conv_w[:, 0, 2]
    out = np.empty((B, T, D), dtype=np.float32)
    for c in range(N_CORES):
        b, h = divmod(c, 2)
        out[b, h * NTOK:(h + 1) * NTOK, :] = results[c]["outT"].T
    for b in range(B):
        y_first_h1 = results[b * 2 + 1]["edges"][:, 0]   # y[b, 1024]
        y_last_h0 = results[b * 2]["edges"][:, 1]        # y[b, 1023]
        out[b, NTOK - 1, :] += w2 * y_first_h1
        out[b, NTOK, :] += w0 * y_last_h0
    return out


_PROGRAM_CACHE = {}


def get_program(reps: int = 1):
    if reps not in _PROGRAM_CACHE:
        _PROGRAM_CACHE[reps] = build_program(reps)
    return _PROGRAM_CACHE[reps]


def kernel(**inputs) -> np.ndarray:
    nc = get_program(1)
    in_maps = prep_in_maps(**inputs)
    res = run_bass_kernel_spmd(nc, in_maps, list(range(N_CORES)))
    return assemble_output(res.results, inputs["conv_w"])

